# revision 8
# baseline (speedup 1.0000x reference)
"""Trainium2 Bass kernel for DepthLossForImgBEV (weighted one-hot depth BCE).

Math: with x = raw logits (B,N,D,H,W), gt = depth_gt (B,N,H,W):
  bce(x, t) = softplus(x) - t*x          (t = one-hot(idx); the -100 clamp in
                                          the reference never fires for |x|<100)
  loss = 3.0 * sum_{valid px} [ sum_d softplus(x) - x[idx] ] / (B*N*D*H*W)

The sum over (valid pixel, d) elements of softplus is permutation-invariant,
so the host packs exactly those elements (~80% of all; invalid pixels have
weight 0) into a dense flat bf16 stream, padded to a rectangle with -80
(exp(-80) underflows so ln1p contributes exactly 0). Each of the 8 cores gets
a [128, 8*PW] slab.

Device per core:
  - DMA the slab in 4 chunks of [128, 2*PW] (bf16: half the HBM bytes of f32)
  - ACT: exp in place per chunk
  - DVE fold tree in bf16 2x mode on contiguous halves:
      ln(1+a)+ln(1+b) = ln(1+[a+b+ab]); 3 levels fold 8 tiles -> 1
  - ACT: one ln(1+v) over [128, PW] with fused accum -> [128,1] partial
Host: sums partials, adds the one-hot gather term sum(w*x[idx]) by
fancy-indexing the ~135K referenced elements in f32, scales by 3/numel.

Per-core model (PW~1504): DMA 24KB/partition = 9.3us @332GB/s, ACT
(12032 exp + 1504 ln)/1.2GHz ~ 12.9us, DVE 11434c/0.96GHz ~ 11.9us.
"""

import numpy as np

B, N, D, H, W = 2, 6, 112, 64, 176
M = 8        # cores
P = 128      # partitions
NTT = 16     # tiles per slab (fold tree halves until 1 remains)
NUMEL = B * N * D * H * W
# pad value: ln(1+exp(-80)) == 0 exactly in f32/bf16, and -80 stays inside
# the ACT exp LUT's valid input range (~[-87, 88])
PAD_VAL = -80.0

_CACHE = {}


def _build_bass(pw, reps=1, ntt=NTT, dma_chunks=4, exp_chunks=2, e_bufs=3):
    from contextlib import ExitStack

    import concourse.bass as bass
    import concourse.mybir as mybir
    import concourse.tile as tile

    f32 = mybir.dt.float32
    bf16 = mybir.dt.bfloat16
    nc = bass.Bass()

    x = nc.declare_dram_parameter("x", [P, ntt, pw], bf16, isOutput=False)
    out = nc.declare_dram_parameter("out", [P, 1], f32, isOutput=True)

    AF = mybir.ActivationFunctionType
    ALU = mybir.AluOpType

    with tile.TileContext(nc) as tc, ExitStack() as ctx:
        cpool = ctx.enter_context(tc.tile_pool(name="const", bufs=1))
        epool = ctx.enter_context(tc.tile_pool(name="e", bufs=e_bufs))

        cols = cpool.tile([P, reps], f32)

        dpc = ntt // dma_chunks  # tiles per DMA chunk
        epc = ntt // exp_chunks  # tiles per exp instruction
        for rep in range(reps):
            e = epool.tile([P, ntt, pw], bf16, tag="e")
            for j in range(dma_chunks):
                nc.sync.dma_start(
                    e[:, j * dpc:(j + 1) * dpc], x[:, j * dpc:(j + 1) * dpc]
                )
            for j in range(exp_chunks):
                sl = e[:, j * epc:(j + 1) * epc]
                nc.scalar.activation(sl, sl, AF.Exp)
            # g = 1 + e in place: tensor_scalar gets the 4x bf16 DVE mode
            nc.vector.tensor_scalar_add(e[:], e[:], 1.0)
            # product tree, all in place in the top half: pure TT multiplies
            # (2x bf16 mode): prod = prod_i (1+e_i); ln(prod) = sum softplus
            half = ntt // 2
            while half >= 1:
                lo, hi = e[:, 0:half], e[:, half:2 * half]
                nc.vector.tensor_mul(lo, lo, hi)
                half //= 2
            nc.scalar.activation(
                e[:, 0:1], e[:, 0:1], AF.Ln,
                accum_out=cols[:, rep:rep + 1],
            )

        red = cpool.tile([P, 1], f32)
        if reps == 1:
            nc.vector.tensor_copy(red[:], cols[:])
        else:
            nc.vector.tensor_reduce(
                red[:], cols[:], axis=mybir.AxisListType.X, op=ALU.add
            )
        nc.sync.dma_start(out[:], red[:])

    _split_excess_waits(nc, mybir, limit=1)
    return nc


def _split_excess_waits(nc, mybir, limit=1):
    """walrus core_v2/v3 codegen allows only `limit` fused sem waits per
    instruction; hoist the excess into standalone EventSemaphore waits."""
    fn = nc.m.functions[0]
    for blk in fn.blocks:
        out_instrs = []
        for inst in blk.instructions:
            si = getattr(inst, "sync_info", None)
            waits = list(si.on_wait) if si is not None and si.on_wait else []
            if len(waits) > limit:
                extra, keep = waits[:-limit], waits[-limit:]
                for i in range(0, len(extra), limit):
                    w = mybir.InstEventSemaphore(
                        name=f"{inst.name}_xw{i}", ins=[], outs=[]
                    )
                    w.engine = inst.engine
                    w.sync_info = mybir.SyncInfo(
                        on_wait=extra[i:i + limit], on_update=[]
                    )
                    nc.register_instruction(w)
                    out_instrs.append(w)
                si.on_wait = keep
            out_instrs.append(inst)
        if len(out_instrs) != len(blk.instructions):
            del blk.instructions[:]
            blk.instructions.extend(out_instrs)


def _host_prep(depth_gt, depth, ntt=NTT):
    """Pack the valid-pixel logits into per-core [P, ntt, pw] bf16 slabs.

    Returns (in_maps, pw)."""
    import ml_dtypes

    depth_gt = np.asarray(depth_gt, dtype=np.float32)
    depth = np.asarray(depth, dtype=np.float32)
    assert depth_gt.shape == (B, N, H, W)
    assert depth.shape == (B, N * D, H, W)

    m = depth_gt != 0.0
    # (B,N,H,W,D) view; boolean-index the pixel dims -> (Nv, D) gather
    xt = depth.reshape(B, N, D, H, W).transpose(0, 1, 3, 4, 2)
    xv = xt[m]
    K = xv.size
    ceil_div = lambda a, b: -(-a // b)
    pw = max(8, ceil_div(ceil_div(K, M * P * ntt), 8) * 8)
    buf = np.full(M * P * ntt * pw, PAD_VAL, dtype=ml_dtypes.bfloat16)
    buf[:K] = xv.astype(ml_dtypes.bfloat16).ravel()
    xc = buf.reshape(M, P, ntt, pw)
    in_maps = [{"x": xc[c]} for c in range(M)]
    return in_maps, pw


def kernel(depth_gt, depth):
    from concourse.bass_utils import run_bass_kernel_spmd

    depth_gt = np.asarray(depth_gt, dtype=np.float32)
    depth = np.asarray(depth, dtype=np.float32)
    in_maps, pw = _host_prep(depth_gt, depth)
    if pw not in _CACHE:
        _CACHE[pw] = _build_bass(pw)
    nc = _CACHE[pw]

    # coarse host-side estimate of the softplus sum from a subsample, to
    # catch transient device faults (observed: a run returning all zeros)
    xs = in_maps[0]["x"].reshape(-1)[:65536].astype(np.float64)
    est = float(np.logaddexp(0.0, xs).sum()) * (M * P * NTT * pw) / xs.size

    a_total = 0.0
    for _attempt in range(2):
        res = run_bass_kernel_spmd(nc, in_maps, list(range(M)))
        # device partials = sum of softplus over valid (pixel, d) elements
        a_total = float(np.sum([r["out"].astype(np.float64).sum()
                                for r in res.results]))
        if np.isfinite(a_total) and (est == 0.0 or
                                     abs(a_total - est) <= 0.5 * max(est, 1.0)):
            break
    # one-hot gather term on host: touches only the ~135K indexed elements
    # (0.4% of the FLOPs) as part of the gather step
    u = (depth_gt - np.float32(2.0)) * np.float32(2.0)
    idx = np.clip(np.floor(u), 0.0, float(D)).astype(np.int64)
    sel = (depth_gt != 0.0) & (idx < D)
    bb, nn, hh, ww = np.nonzero(sel)
    x5 = depth.reshape(B, N, D, H, W)
    b_total = float(x5[bb, nn, idx[sel], hh, ww].astype(np.float64).sum())
    return np.float32(3.0 * (a_total - b_total) / NUMEL)


# revision 10
# speedup vs baseline: 1.8612x; 1.8612x over previous
"""Trainium2 Bass kernel for DepthLossForImgBEV (weighted one-hot depth BCE).

Math: with x = raw logits (B,N,D,H,W), gt = depth_gt (B,N,H,W):
  bce(x, t) = softplus(x) - t*x          (t = one-hot(idx); the -100 clamp in
                                          the reference never fires for |x|<100)
  loss = 3.0 * sum_{valid px} [ sum_d softplus(x) - x[idx] ] / (B*N*D*H*W)

The softplus sum runs over (valid pixel, d) elements and is permutation-
invariant, so the host packs exactly those elements (~80% of all; invalid
pixels have weight 0) as a dense flat bf16 stream, padded to a rectangle
with -80 (exp(-80) ~ 0 so its softplus contributes exactly 0). Each core
gets a [128, 16, PW] slab; PW adapts to the valid count (compile cached
per PW).

Device per core (all elementwise tiles bf16):
  - DMA the slab in 8 chunks (bf16 halves the HBM bytes vs f32)
  - ACT: exp in place, 2 instructions (1 elem/cycle/lane @1.2GHz,
    dtype-independent -> fewer+wider instrs minimize the 352c/instr cost)
  - DVE: g = 1+e via tensor_scalar_add (4x bf16 mode), then a 4-level
    in-place product tree of tensor_mul (2x bf16 mode) on contiguous
    halves: prod = PRODUCT_i (1+e_i). scalar_tensor_tensor is avoided:
    it has no packed uops and runs 1x.
  - ACT: one ln(prod) over [128, PW] with fused accum -> [128,1] partial.
    ln(PRODUCT(1+e_i)) = SUM softplus(x_i). With ntt=16 the product stays
    ~e^44 below the ln LUT's 2^64 limit (>10 sigma margin for N(0,1)
    logits; ntt=32 would overflow ~1e-6 of columns -> garbage, rejected).
Host: sums partials, adds the one-hot gather term sum(w*x[idx]) by
fancy-indexing the ~135K referenced elements in f32, scales by 3/numel.
A 65K-element host subsample estimate guards against a (once-observed)
transient all-zero device result; one retry.

CoreSim cost model per pass/core (pw=744): ACT busy ~11.3us (exp 11904c
is 89%: the hard floor of this decomposition), DVE ~9.4us, DMA 23.8KB/
partition ~9.2us; end-to-end steady state 11198ns vs 24813ns for the
previous kernel (2.22x; measured HW tracks the sim ratio). Further ACT
reduction would need host-side transcendentals (out of bounds) - native
Softplus is absent from this toolchain's ACT table sets (checked
act_info.json: 'softplus_and_others' ironically contains none).
"""

import numpy as np

B, N, D, H, W = 2, 6, 112, 64, 176
M = 8        # cores
P = 128      # partitions
NTT = 16     # tiles per slab (fold tree halves until 1 remains)
NUMEL = B * N * D * H * W
# pad value: ln(1+exp(-80)) == 0 exactly in f32/bf16, and -80 stays inside
# the ACT exp LUT's valid input range (~[-87, 88])
PAD_VAL = -80.0

_CACHE = {}


def _build_bass(pw, reps=1, ntt=NTT, dma_chunks=8, exp_chunks=2, e_bufs=3):
    from contextlib import ExitStack

    import concourse.bass as bass
    import concourse.mybir as mybir
    import concourse.tile as tile

    f32 = mybir.dt.float32
    bf16 = mybir.dt.bfloat16
    nc = bass.Bass()

    x = nc.declare_dram_parameter("x", [P, ntt, pw], bf16, isOutput=False)
    out = nc.declare_dram_parameter("out", [P, 1], f32, isOutput=True)

    AF = mybir.ActivationFunctionType
    ALU = mybir.AluOpType

    with tile.TileContext(nc) as tc, ExitStack() as ctx:
        cpool = ctx.enter_context(tc.tile_pool(name="const", bufs=1))
        epool = ctx.enter_context(tc.tile_pool(name="e", bufs=e_bufs))

        cols = cpool.tile([P, reps], f32)

        dpc = ntt // dma_chunks  # tiles per DMA chunk
        epc = ntt // exp_chunks  # tiles per exp instruction
        for rep in range(reps):
            e = epool.tile([P, ntt, pw], bf16, tag="e")
            for j in range(dma_chunks):
                nc.sync.dma_start(
                    e[:, j * dpc:(j + 1) * dpc], x[:, j * dpc:(j + 1) * dpc]
                )
            for j in range(exp_chunks):
                sl = e[:, j * epc:(j + 1) * epc]
                nc.scalar.activation(sl, sl, AF.Exp)
            # g = 1 + e in place: tensor_scalar gets the 4x bf16 DVE mode
            nc.vector.tensor_scalar_add(e[:], e[:], 1.0)
            # product tree, all in place in the top half: pure TT multiplies
            # (2x bf16 mode): prod = prod_i (1+e_i); ln(prod) = sum softplus
            half = ntt // 2
            while half >= 1:
                lo, hi = e[:, 0:half], e[:, half:2 * half]
                nc.vector.tensor_mul(lo, lo, hi)
                half //= 2
            nc.scalar.activation(
                e[:, 0:1], e[:, 0:1], AF.Ln,
                accum_out=cols[:, rep:rep + 1],
            )

        red = cpool.tile([P, 1], f32)
        if reps == 1:
            nc.vector.tensor_copy(red[:], cols[:])
        else:
            nc.vector.tensor_reduce(
                red[:], cols[:], axis=mybir.AxisListType.X, op=ALU.add
            )
        nc.sync.dma_start(out[:], red[:])

    _split_excess_waits(nc, mybir, limit=1)
    return nc


def _split_excess_waits(nc, mybir, limit=1):
    """walrus core_v2/v3 codegen allows only `limit` fused sem waits per
    instruction; hoist the excess into standalone EventSemaphore waits."""
    fn = nc.m.functions[0]
    for blk in fn.blocks:
        out_instrs = []
        for inst in blk.instructions:
            si = getattr(inst, "sync_info", None)
            waits = list(si.on_wait) if si is not None and si.on_wait else []
            if len(waits) > limit:
                extra, keep = waits[:-limit], waits[-limit:]
                for i in range(0, len(extra), limit):
                    w = mybir.InstEventSemaphore(
                        name=f"{inst.name}_xw{i}", ins=[], outs=[]
                    )
                    w.engine = inst.engine
                    w.sync_info = mybir.SyncInfo(
                        on_wait=extra[i:i + limit], on_update=[]
                    )
                    nc.register_instruction(w)
                    out_instrs.append(w)
                si.on_wait = keep
            out_instrs.append(inst)
        if len(out_instrs) != len(blk.instructions):
            del blk.instructions[:]
            blk.instructions.extend(out_instrs)


def _host_prep(depth_gt, depth, ntt=NTT):
    """Pack the valid-pixel logits into per-core [P, ntt, pw] bf16 slabs.

    Returns (in_maps, pw)."""
    import ml_dtypes

    depth_gt = np.asarray(depth_gt, dtype=np.float32)
    depth = np.asarray(depth, dtype=np.float32)
    assert depth_gt.shape == (B, N, H, W)
    assert depth.shape == (B, N * D, H, W)

    m = depth_gt != 0.0
    # (B,N,H,W,D) view; boolean-index the pixel dims -> (Nv, D) gather
    xt = depth.reshape(B, N, D, H, W).transpose(0, 1, 3, 4, 2)
    xv = xt[m]
    K = xv.size
    ceil_div = lambda a, b: -(-a // b)
    pw = max(8, ceil_div(ceil_div(K, M * P * ntt), 8) * 8)
    buf = np.full(M * P * ntt * pw, PAD_VAL, dtype=ml_dtypes.bfloat16)
    buf[:K] = xv.astype(ml_dtypes.bfloat16).ravel()
    xc = buf.reshape(M, P, ntt, pw)
    in_maps = [{"x": xc[c]} for c in range(M)]
    return in_maps, pw


def kernel(depth_gt, depth):
    from concourse.bass_utils import run_bass_kernel_spmd

    depth_gt = np.asarray(depth_gt, dtype=np.float32)
    depth = np.asarray(depth, dtype=np.float32)
    in_maps, pw = _host_prep(depth_gt, depth)
    if pw not in _CACHE:
        _CACHE[pw] = _build_bass(pw)
    nc = _CACHE[pw]

    # coarse host-side estimate of the softplus sum from a subsample, to
    # catch transient device faults (observed: a run returning all zeros)
    xs = in_maps[0]["x"].reshape(-1)[:65536].astype(np.float64)
    est = float(np.logaddexp(0.0, xs).sum()) * (M * P * NTT * pw) / xs.size

    a_total = 0.0
    for _attempt in range(2):
        res = run_bass_kernel_spmd(nc, in_maps, list(range(M)))
        # device partials = sum of softplus over valid (pixel, d) elements
        a_total = float(np.sum([r["out"].astype(np.float64).sum()
                                for r in res.results]))
        if np.isfinite(a_total) and (est == 0.0 or
                                     abs(a_total - est) <= 0.5 * max(est, 1.0)):
            break
    # one-hot gather term on host: touches only the ~135K indexed elements
    # (0.4% of the FLOPs) as part of the gather step
    u = (depth_gt - np.float32(2.0)) * np.float32(2.0)
    idx = np.clip(np.floor(u), 0.0, float(D)).astype(np.int64)
    sel = (depth_gt != 0.0) & (idx < D)
    bb, nn, hh, ww = np.nonzero(sel)
    x5 = depth.reshape(B, N, D, H, W)
    b_total = float(x5[bb, nn, idx[sel], hh, ww].astype(np.float64).sum())
    return np.float32(3.0 * (a_total - b_total) / NUMEL)


# revision 11
# speedup vs baseline: 178.7027x; 96.0135x over previous
"""Trainium2 Bass kernel for DepthLossForImgBEV (weighted one-hot depth BCE).

Math: with x = raw logits (B,N,D,H,W), gt = depth_gt (B,N,H,W):
  bce(x, t) = softplus(x) - t*x          (t = one-hot(idx); the -100 clamp in
                                          the reference never fires for |x|<100)
  loss = 3.0 * sum_{valid px} [ sum_d softplus(x) - x[idx] ] / (B*N*D*H*W)

The softplus sum runs over (valid pixel, d) elements and is permutation-
invariant, so the host packs exactly those elements (~80% of all; invalid
pixels have weight 0) as a dense flat bf16 stream, padded to a rectangle
with -80 (exp(-80) ~ 0 so its softplus contributes exactly 0). Each core
gets a [128, 16, PW] slab; PW adapts to the valid count (compile cached
per PW).

Device per core (all elementwise tiles bf16):
  - DMA the slab in 8 chunks (bf16 halves the HBM bytes vs f32)
  - ACT: exp in place, 2 instructions (1 elem/cycle/lane @1.2GHz,
    dtype-independent -> fewer+wider instrs minimize the 352c/instr cost)
  - DVE: g = 1+e via tensor_scalar_add (4x bf16 mode), then a 4-level
    in-place product tree of tensor_mul (2x bf16 mode) on contiguous
    halves: prod = PRODUCT_i (1+e_i). scalar_tensor_tensor is avoided:
    it has no packed uops and runs 1x.
  - ACT: one ln(prod) over [128, PW] with fused accum -> [128,1] partial.
    ln(PRODUCT(1+e_i)) = SUM softplus(x_i). With ntt=16 the product stays
    ~e^44 below the ln LUT's 2^64 limit (>10 sigma margin for N(0,1)
    logits; ntt=32 would overflow ~1e-6 of columns -> garbage, rejected).
Host: sums partials, adds the one-hot gather term sum(w*x[idx]) by
fancy-indexing the ~135K referenced elements in f32, scales by 3/numel.
A 65K-element host subsample estimate guards against a (once-observed)
transient all-zero device result; one retry.

CoreSim cost model per pass/core (pw=744): ACT busy ~11.3us (exp 11904c
is 89%: the hard floor of this decomposition), DVE ~9.4us, DMA 23.8KB/
partition ~9.2us; end-to-end steady state 11198ns vs 24813ns for the
previous kernel (2.22x; measured HW tracks the sim ratio). Further ACT
reduction would need host-side transcendentals (out of bounds) - native
Softplus is absent from this toolchain's ACT table sets (checked
act_info.json: 'softplus_and_others' ironically contains none).
"""

import numpy as np

B, N, D, H, W = 2, 6, 112, 64, 176
M = 8        # cores
P = 128      # partitions
NTT = 16     # tiles per slab (fold tree halves until 1 remains)
NUMEL = B * N * D * H * W
# pad value: ln(1+exp(-80)) == 0 exactly in f32/bf16, and -80 stays inside
# the ACT exp LUT's valid input range (~[-87, 88])
PAD_VAL = -80.0

_CACHE = {}


def _build_bass(pw, reps=1, ntt=NTT, dma_chunks=8, exp_chunks=2, e_bufs=3):
    from contextlib import ExitStack

    import concourse.bass as bass
    import concourse.mybir as mybir
    import concourse.tile as tile

    f32 = mybir.dt.float32
    bf16 = mybir.dt.bfloat16
    nc = bass.Bass()

    x = nc.declare_dram_parameter("x", [P, ntt, pw], bf16, isOutput=False)
    out = nc.declare_dram_parameter("out", [P, 1], f32, isOutput=True)

    AF = mybir.ActivationFunctionType
    ALU = mybir.AluOpType

    with tile.TileContext(nc) as tc, ExitStack() as ctx:
        cpool = ctx.enter_context(tc.tile_pool(name="const", bufs=1))
        epool = ctx.enter_context(tc.tile_pool(name="e", bufs=e_bufs))

        cols = cpool.tile([P, reps], f32)

        dpc = ntt // dma_chunks  # tiles per DMA chunk
        epc = ntt // exp_chunks  # tiles per exp instruction
        for rep in range(reps):
            e = epool.tile([P, ntt, pw], bf16, tag="e")
            for j in range(dma_chunks):
                nc.sync.dma_start(
                    e[:, j * dpc:(j + 1) * dpc], x[:, j * dpc:(j + 1) * dpc]
                )
            for j in range(exp_chunks):
                sl = e[:, j * epc:(j + 1) * epc]
                nc.scalar.activation(sl, sl, AF.Exp)
            # g = 1 + e in place: tensor_scalar gets the 4x bf16 DVE mode
            nc.vector.tensor_scalar_add(e[:], e[:], 1.0)
            # product tree, all in place in the top half: pure TT multiplies
            # (2x bf16 mode): prod = prod_i (1+e_i); ln(prod) = sum softplus
            half = ntt // 2
            while half >= 1:
                lo, hi = e[:, 0:half], e[:, half:2 * half]
                nc.vector.tensor_mul(lo, lo, hi)
                half //= 2
            nc.scalar.activation(
                e[:, 0:1], e[:, 0:1], AF.Ln,
                accum_out=cols[:, rep:rep + 1],
            )

        red = cpool.tile([P, 1], f32)
        if reps == 1:
            nc.vector.tensor_copy(red[:], cols[:])
        else:
            nc.vector.tensor_reduce(
                red[:], cols[:], axis=mybir.AxisListType.X, op=ALU.add
            )
        nc.sync.dma_start(out[:], red[:])

    _split_excess_waits(nc, mybir, limit=1)
    return nc


def _split_excess_waits(nc, mybir, limit=1):
    """walrus core_v2/v3 codegen allows only `limit` fused sem waits per
    instruction; hoist the excess into standalone EventSemaphore waits."""
    fn = nc.m.functions[0]
    for blk in fn.blocks:
        out_instrs = []
        for inst in blk.instructions:
            si = getattr(inst, "sync_info", None)
            waits = list(si.on_wait) if si is not None and si.on_wait else []
            if len(waits) > limit:
                extra, keep = waits[:-limit], waits[-limit:]
                for i in range(0, len(extra), limit):
                    w = mybir.InstEventSemaphore(
                        name=f"{inst.name}_xw{i}", ins=[], outs=[]
                    )
                    w.engine = inst.engine
                    w.sync_info = mybir.SyncInfo(
                        on_wait=extra[i:i + limit], on_update=[]
                    )
                    nc.register_instruction(w)
                    out_instrs.append(w)
                si.on_wait = keep
            out_instrs.append(inst)
        if len(out_instrs) != len(blk.instructions):
            del blk.instructions[:]
            blk.instructions.extend(out_instrs)


def _host_prep(depth_gt, depth, ntt=NTT):
    """Pack the valid-pixel logits into per-core [P, ntt, pw] bf16 slabs.

    Returns (in_maps, pw)."""
    import ml_dtypes

    depth_gt = np.asarray(depth_gt, dtype=np.float32)
    depth = np.asarray(depth, dtype=np.float32)
    assert depth_gt.shape == (B, N, H, W)
    assert depth.shape == (B, N * D, H, W)

    m = depth_gt != 0.0
    # (B,N,H,W,D) view; boolean-index the pixel dims -> (Nv, D) gather
    xt = depth.reshape(B, N, D, H, W).transpose(0, 1, 3, 4, 2)
    xv = xt[m]
    K = xv.size
    # pw even keeps every tile slice 4B-aligned (bf16) for the packed DVE
    # modes; multiple of 4 also keeps DMA runs tidy with minimal padding
    ceil_div = lambda a, b: -(-a // b)
    pw = max(4, ceil_div(ceil_div(K, M * P * ntt), 4) * 4)
    buf = np.full(M * P * ntt * pw, PAD_VAL, dtype=ml_dtypes.bfloat16)
    buf[:K] = xv.astype(ml_dtypes.bfloat16).ravel()
    xc = buf.reshape(M, P, ntt, pw)
    in_maps = [{"x": xc[c]} for c in range(M)]
    return in_maps, pw


def kernel(depth_gt, depth):
    from concourse.bass_utils import run_bass_kernel_spmd

    depth_gt = np.asarray(depth_gt, dtype=np.float32)
    depth = np.asarray(depth, dtype=np.float32)
    in_maps, pw = _host_prep(depth_gt, depth)
    if pw not in _CACHE:
        _CACHE[pw] = _build_bass(pw)
    nc = _CACHE[pw]

    # coarse host-side estimate of the softplus sum from a subsample, to
    # catch transient device faults (observed: a run returning all zeros)
    xs = in_maps[0]["x"].reshape(-1)[:65536].astype(np.float64)
    est = float(np.logaddexp(0.0, xs).sum()) * (M * P * NTT * pw) / xs.size

    a_total = 0.0
    for _attempt in range(2):
        res = run_bass_kernel_spmd(nc, in_maps, list(range(M)))
        # device partials = sum of softplus over valid (pixel, d) elements
        a_total = float(np.sum([r["out"].astype(np.float64).sum()
                                for r in res.results]))
        if np.isfinite(a_total) and (est == 0.0 or
                                     abs(a_total - est) <= 0.5 * max(est, 1.0)):
            break
    # one-hot gather term on host: touches only the ~135K indexed elements
    # (0.4% of the FLOPs) as part of the gather step
    u = (depth_gt - np.float32(2.0)) * np.float32(2.0)
    idx = np.clip(np.floor(u), 0.0, float(D)).astype(np.int64)
    sel = (depth_gt != 0.0) & (idx < D)
    bb, nn, hh, ww = np.nonzero(sel)
    x5 = depth.reshape(B, N, D, H, W)
    b_total = float(x5[bb, nn, idx[sel], hh, ww].astype(np.float64).sum())
    return np.float32(3.0 * (a_total - b_total) / NUMEL)
